# revision 27
# baseline (speedup 1.0000x reference)
"""Trainium2 Bass kernel: per-row codon histogram + tissue embedding + 2-layer MLP.

Model (matches the jax reference):
  freq[b]  = hist(rna_data[b], bins 1..64) / n_nonzero[b]
  emb[b]   = renorm(emb_table)[tissue_id[b]]
  x        = [emb | freq]  (128 features)
  h        = relu(x @ W1.T + b1)
  out[b]   = sigmoid(h @ W2.T + b2)

Mapping (v4) — three-way bin split to balance engines:
  - bins 1..41  (PE):  data staged transposed [positions, rows]; DVE builds
    (data == c) bf16 masks at 4x; PE reduces masks along positions via a
    sliding-window selector stationary into PSUM row c-1, accumulating
    across position chunks. One PSUM bank per 512-row group.
  - bins 42..44 (DVE): fused is_equal+accumulate on row-major data (1x).
  - bins 45..64 (ACT): hockey-stick trick on row-major data: T(c) =
    sum relu(v - c) per row via activation accum; exact counts via second
    differences. fp32 arithmetic is exact (values < 2^24).
  - MLP: embedding renorm + W1a + b1 host-folded into G2[30,256]; one-hot
    (tissue) + freq share one PE stationary; W2 + sigmoid on PE/ACT.
"""

import os
import sys

import numpy as np

try:
    import concourse  # noqa: F401
except ImportError:  # pragma: no cover
    for _p in ("/opt/trn_rl_repo", "/root/.axon_site/_ro/trn_rl_repo"):
        if os.path.isdir(_p):
            sys.path.insert(0, _p)
            break

from contextlib import ExitStack

import concourse.bass as bass
import concourse.tile as tile
from concourse import bacc, mybir
from concourse import masks as cmasks
from concourse.bass_utils import run_bass_kernel_spmd

N_CORES = 8
B, L = 16384, 2048
RPC = B // N_CORES          # rows per core
NT = RPC // 128             # 128-row tiles per core (16)
NCH = L // 128              # 128-position chunks (16)
NBINS = 64                  # histogram bins 1..64 (bin 0 = padding, dropped)
NTIS = 30                   # tissue vocabulary
NFEAT = NTIS + NBINS        # 94 combined one-hot + freq features
NFEAT_PAD = 96
GROUP = 512                 # rows per PE moving block
NGRP = RPC // GROUP         # 4
Z_PE = 42                   # bins 1..Z_PE via PE mask-reduce
NDVE = 3                    # bins Z_PE+1..Z_PE+NDVE via DVE is_eq+accum
ABASE = Z_PE + NDVE         # 44: ACT bins ABASE+1..64
NACT = NBINS - ABASE        # 20
SCHW = 4                    # chunks per supertile
NSCH = NCH // SCHW          # 4

f32 = mybir.dt.float32
bf16 = mybir.dt.bfloat16
i32 = mybir.dt.int32
OP = mybir.AluOpType
AF = mybir.ActivationFunctionType

_CACHED = {}


def _build_program():
    nc = bacc.Bacc(
        "TRN2",
        target_bir_lowering=False,
        debug=False,
        enable_asserts=False,
        num_devices=N_CORES,
    )

    # rnat = transposed shard [L positions, RPC rows]; rna = row-major shard
    rnat = nc.dram_tensor("rnat", [L, RPC], i32, kind="ExternalInput").ap()
    rna = nc.dram_tensor("rna", [RPC, L], i32, kind="ExternalInput").ap()
    tis = nc.dram_tensor("tis", [128, NT], i32, kind="ExternalInput").ap()
    s0 = nc.dram_tensor("s0", [NFEAT_PAD, 128], f32, kind="ExternalInput").ap()
    s1 = nc.dram_tensor("s1", [NFEAT_PAD, 128], f32, kind="ExternalInput").ap()
    w2a = nc.dram_tensor("w2a", [128, 1], f32, kind="ExternalInput").ap()
    w2b = nc.dram_tensor("w2b", [128, 1], f32, kind="ExternalInput").ap()
    b2v = nc.dram_tensor("b2v", [1, 1], f32, kind="ExternalInput").ap()
    out = nc.dram_tensor("out", [NT, 128], f32, kind="ExternalOutput").ap()

    with tile.TileContext(nc) as tc, ExitStack() as ctx:
        const = ctx.enter_context(tc.tile_pool(name="const", bufs=1))

        ident = const.tile([128, 128], f32)
        cmasks.make_identity(nc, ident[:])
        # zsel: zeros except column 64 all-ones. Bin c's stationary is the
        # sliding view zsel[:, 65-c : 129-c] -> routes the mask-sum into
        # output partition row c-1.
        zsel = const.tile([128, 129], bf16)
        nc.vector.memset(zsel[:], 0.0)
        nc.vector.memset(zsel[:, 64:65], 1.0)

        iota_i = const.tile([128, 32], i32)
        nc.gpsimd.iota(iota_i[:], [[1, 32]], channel_multiplier=0)
        iota_f = const.tile([128, 32], f32)
        nc.vector.tensor_copy(iota_f[:], iota_i[:])

        s0_sb = const.tile([NFEAT_PAD, 128], f32)
        nc.sync.dma_start(s0_sb[:], s0)
        s1_sb = const.tile([NFEAT_PAD, 128], f32)
        nc.sync.dma_start(s1_sb[:], s1)
        w2a_sb = const.tile([128, 1], f32)
        nc.sync.dma_start(w2a_sb[:], w2a)
        w2b_sb = const.tile([128, 1], f32)
        nc.sync.dma_start(w2b_sb[:], w2b)
        b2_sb = const.tile([1, 1], f32)
        nc.sync.dma_start(b2_sb[:], b2v)
        tis_i = const.tile([128, NT], i32)
        nc.sync.dma_start(tis_i[:], tis)
        tis_f = const.tile([128, NT], f32)
        nc.vector.tensor_copy(tis_f[:], tis_i[:])

        # results staged across phases
        nacts = const.tile([128, NT * NACT], f32)   # ACT bins per tile
        naccs = const.tile([128, NT * NDVE], f32)   # DVE-accum bins per tile
        htt_all = const.tile([Z_PE, RPC], f32)      # PE histT staging
        # bias column k = -(ABASE + k) for the relu-T passes
        tbias = const.tile([128, NACT], f32)
        nc.vector.tensor_scalar(
            tbias[:], iota_f[:, 0:NACT], float(ABASE), -1.0, OP.add, OP.mult
        )

        rnat_pool = ctx.enter_context(tc.tile_pool(name="rnat", bufs=2))
        vbf_pool = ctx.enter_context(tc.tile_pool(name="vbf", bufs=2))
        mask_pool = ctx.enter_context(tc.tile_pool(name="mask", bufs=3))
        rowr_pool = ctx.enter_context(tc.tile_pool(name="rowr", bufs=2))
        rowb_pool = ctx.enter_context(tc.tile_pool(name="rowb", bufs=2))
        scr_pool = ctx.enter_context(tc.tile_pool(name="scr", bufs=2))
        hist_pool = ctx.enter_context(tc.tile_pool(name="hist", bufs=3))
        xoh_pool = ctx.enter_context(tc.tile_pool(name="xoh", bufs=3))
        small_pool = ctx.enter_context(tc.tile_pool(name="small", bufs=3))
        mlp_pool = ctx.enter_context(tc.tile_pool(name="mlp", bufs=3))

        # --- Phase 1: sweeps ---
        with tc.tile_pool(name="sweeppsum", bufs=1, space="PSUM") as sweep_psum:
            hps = []
            for bnk in range(NGRP):
                hb = sweep_psum.tile([64, GROUP], f32, tag=f"hb{bnk}")
                hps.append(hb)

            for sch in range(NSCH):
                rt = rnat_pool.tile([128, SCHW, RPC], i32)
                src = rnat[bass.ts(sch, 128 * SCHW), :].rearrange(
                    "(a b) c -> b a c", a=SCHW
                )
                nc.sync.dma_start(rt[:], src)
                vb = vbf_pool.tile([128, SCHW * RPC], bf16)
                nc.vector.tensor_copy(vb[:], rt[:].rearrange("b a c -> b (a c)"))
                for c in range(1, Z_PE + 1):
                    mask = mask_pool.tile([128, SCHW * RPC], bf16)
                    nc.vector.tensor_scalar(
                        mask[:], vb[:], float(c), None, OP.is_equal
                    )
                    for h in range(SCHW):
                        for g in range(NGRP):
                            nc.tensor.matmul(
                                hps[g][:],
                                zsel[:, 65 - c : 129 - c],
                                mask[
                                    :,
                                    h * RPC + g * GROUP : h * RPC + (g + 1) * GROUP,
                                ],
                                start=(sch == 0 and h == 0 and c == 1),
                                stop=(
                                    sch == NSCH - 1
                                    and h == SCHW - 1
                                    and c == Z_PE
                                ),
                            )

                # row-major work for SCHW row-tiles, interleaved for overlap
                for t in range(SCHW * sch, SCHW * (sch + 1)):
                    rtr = rowr_pool.tile([128, L], i32)
                    nc.sync.dma_start(rtr[:], rna[bass.ts(t, 128), :])
                    # DVE fused compare+accumulate bins (bf16 cast first:
                    # the accum-reduce variant rejects int32 inputs)
                    vbr = rowb_pool.tile([128, L], bf16)
                    nc.vector.tensor_copy(vbr[:], rtr[:])
                    for j in range(NDVE):
                        scr2 = scr_pool.tile([128, L], bf16, tag="scr2")
                        nc.vector.tensor_scalar(
                            scr2[:],
                            vbr[:],
                            float(Z_PE + 1 + j),
                            None,
                            OP.is_equal,
                            OP.add,
                            accum_out=naccs[:, t * NDVE + j : t * NDVE + j + 1],
                        )
                    # ACT relu-T passes (reads int32 directly)
                    tt = small_pool.tile([128, NACT + 2], f32, tag="tt")
                    nc.vector.memset(tt[:, NACT : NACT + 2], 0.0)
                    for k in range(NACT):
                        scr = scr_pool.tile([128, L], bf16, tag="scr")
                        nc.scalar.activation(
                            scr[:],
                            vbr[:],
                            AF.Relu,
                            bias=tbias[:, k : k + 1],
                            accum_out=tt[:, k : k + 1],
                        )
                    cdif = small_pool.tile([128, NACT + 1], f32, tag="cdif")
                    nc.vector.tensor_tensor(
                        cdif[:], tt[:, 0 : NACT + 1], tt[:, 1 : NACT + 2], OP.subtract
                    )
                    nc.vector.tensor_tensor(
                        nacts[:, t * NACT : (t + 1) * NACT],
                        cdif[:, 0:NACT],
                        cdif[:, 1 : NACT + 1],
                        OP.subtract,
                    )

            # drain PE hist PSUM to SBUF so the sweep banks can be reused
            for t in range(NT):
                g = t // 4
                sl = t % 4
                nc.vector.tensor_copy(
                    htt_all[:, bass.ts(t, 128)], hps[g][0:Z_PE, bass.ts(sl, 128)]
                )

        # --- Phase 2+3: assemble hist, normalize, MLP ---
        with (
            tc.tile_pool(name="trpsum", bufs=2, space="PSUM") as tr_psum,
            tc.tile_pool(name="mlppsum", bufs=2, space="PSUM") as mlp_psum,
        ):
            for t in range(NT):
                hps_t = tr_psum.tile([128, Z_PE], f32, tag="ht")
                nc.tensor.transpose(
                    hps_t[:], htt_all[:, bass.ts(t, 128)], ident[0:Z_PE, 0:Z_PE]
                )
                hist = hist_pool.tile([128, NBINS], f32)
                nc.vector.tensor_copy(hist[:, 0:Z_PE], hps_t[:])
                nc.vector.tensor_copy(
                    hist[:, Z_PE:ABASE], naccs[:, t * NDVE : (t + 1) * NDVE]
                )
                nc.vector.tensor_copy(
                    hist[:, ABASE:NBINS], nacts[:, t * NACT : (t + 1) * NACT]
                )

                xoh = xoh_pool.tile([128, NFEAT_PAD], f32)
                nc.vector.tensor_scalar(
                    xoh[:, 0:NTIS],
                    iota_f[:, 0:NTIS],
                    tis_f[:, t : t + 1],
                    None,
                    OP.is_equal,
                )
                nvec = small_pool.tile([128, 1], f32, tag="nvec")
                nc.vector.tensor_reduce(
                    nvec[:], hist[:], mybir.AxisListType.X, OP.add
                )
                rec = small_pool.tile([128, 1], f32, tag="rec")
                nc.vector.reciprocal(rec[:], nvec[:])
                nc.vector.tensor_scalar(
                    xoh[:, NTIS:NFEAT], hist[:], rec[:], None, OP.mult
                )
                nc.vector.memset(xoh[:, NFEAT:NFEAT_PAD], 0.0)

                xt_ps = tr_psum.tile([128, 128], f32, tag="xt")
                nc.tensor.transpose(xt_ps[0:NFEAT_PAD, :], xoh[:], ident[:])
                xt_sb = mlp_pool.tile([NFEAT_PAD, 128], f32, tag="xtsb")
                nc.vector.tensor_copy(xt_sb[:], xt_ps[0:NFEAT_PAD, :])

                hh = mlp_psum.tile([128, 256], f32, tag="hh")
                nc.tensor.matmul(
                    hh[:, 0:128], s0_sb[:], xt_sb[:], start=True, stop=False
                )
                nc.tensor.matmul(
                    hh[:, 128:256], s1_sb[:], xt_sb[:], start=False, stop=True
                )

                hr0 = mlp_pool.tile([128, 128], f32, tag="hr0")
                nc.vector.tensor_scalar(hr0[:], hh[:, 0:128], 0.0, None, OP.max)
                hr1 = mlp_pool.tile([128, 128], f32, tag="hr1")
                nc.vector.tensor_scalar(hr1[:], hh[:, 128:256], 0.0, None, OP.max)

                yps = mlp_psum.tile([1, 128], f32, tag="y")
                nc.tensor.matmul(yps[:], w2a_sb[:], hr0[:], start=True, stop=False)
                nc.tensor.matmul(yps[:], w2b_sb[:], hr1[:], start=False, stop=True)

                ysb = mlp_pool.tile([1, 128], f32, tag="ysb")
                nc.scalar.activation(ysb[:], yps[:], AF.Sigmoid, bias=b2_sb[:])
                nc.sync.dma_start(out[t : t + 1, :], ysb[:])

    nc.compile()
    return nc


def _host_tables(emb_table, W1, b1, W2, b2):
    emb_table = np.asarray(emb_table, np.float32)
    W1 = np.asarray(W1, np.float32)
    b1 = np.asarray(b1, np.float32)
    W2 = np.asarray(W2, np.float32)
    b2 = np.asarray(b2, np.float32)

    norm = np.linalg.norm(emb_table, axis=1, keepdims=True)
    scale = np.where(norm > 1.0, 1.0 / (norm + 1e-7), 1.0).astype(np.float32)
    embr = emb_table * scale                      # [30, 64]

    W1a = W1[:, :NBINS]                           # [256, 64] (emb features)
    W1b = W1[:, NBINS:]                           # [256, 64] (freq features)
    G2 = embr @ W1a.T + b1[None, :]               # [30, 256]

    s_halves = []
    for h in range(2):
        S = np.zeros((NFEAT_PAD, 128), np.float32)
        S[0:NTIS, :] = G2[:, 128 * h : 128 * (h + 1)]
        S[NTIS:NFEAT, :] = W1b[128 * h : 128 * (h + 1), :].T
        s_halves.append(S)
    w2a = np.ascontiguousarray(W2[0, 0:128].reshape(128, 1))
    w2b = np.ascontiguousarray(W2[0, 128:256].reshape(128, 1))
    b2v = np.array([[float(b2.reshape(-1)[0])]], np.float32)
    return s_halves[0], s_halves[1], w2a, w2b, b2v


def build_in_maps(rna_data, tissue_id, emb_table, W1, b1, W2, b2):
    rna_data = np.asarray(rna_data, np.int32)
    tissue_id = np.asarray(tissue_id, np.int32).reshape(-1)
    s0, s1, w2a, w2b, b2v = _host_tables(emb_table, W1, b1, W2, b2)

    in_maps = []
    for c in range(N_CORES):
        rsh = np.ascontiguousarray(rna_data[c * RPC : (c + 1) * RPC])
        tsh = tissue_id[c * RPC : (c + 1) * RPC]
        tis_m = np.ascontiguousarray(tsh.reshape(NT, 128).T.astype(np.int32))
        in_maps.append(
            {
                "rnat": np.ascontiguousarray(rsh.T),
                "rna": rsh,
                "tis": tis_m,
                "s0": s0,
                "s1": s1,
                "w2a": w2a,
                "w2b": w2b,
                "b2v": b2v,
            }
        )
    return in_maps


def get_program():
    if "nc" not in _CACHED:
        _CACHED["nc"] = _build_program()
    return _CACHED["nc"]


def kernel(rna_data, tissue_id, emb_table, W1, b1, W2, b2, **run_kwargs):
    nc = get_program()
    in_maps = build_in_maps(rna_data, tissue_id, emb_table, W1, b1, W2, b2)
    res = run_bass_kernel_spmd(nc, in_maps, list(range(N_CORES)), **run_kwargs)
    outs = [res.results[c]["out"].reshape(RPC) for c in range(N_CORES)]
    full = np.concatenate(outs).reshape(B, 1).astype(np.float32)
    if run_kwargs:
        return full, res
    return full


# revision 28
# speedup vs baseline: 1.1429x; 1.1429x over previous
"""Trainium2 Bass kernel: per-row codon histogram + tissue embedding + 2-layer MLP.

Model (matches the jax reference):
  freq[b]  = hist(rna_data[b], bins 1..64) / n_nonzero[b]
  emb[b]   = renorm(emb_table)[tissue_id[b]]
  x        = [emb | freq]  (128 features)
  h        = relu(x @ W1.T + b1)
  out[b]   = sigmoid(h @ W2.T + b2)

Mapping (v4) — three-way bin split to balance engines:
  - bins 1..41  (PE):  data staged transposed [positions, rows]; DVE builds
    (data == c) bf16 masks at 4x; PE reduces masks along positions via a
    sliding-window selector stationary into PSUM row c-1, accumulating
    across position chunks. One PSUM bank per 512-row group.
  - bins 42..44 (DVE): fused is_equal+accumulate on row-major data (1x).
  - bins 45..64 (ACT): hockey-stick trick on row-major data: T(c) =
    sum relu(v - c) per row via activation accum; exact counts via second
    differences. fp32 arithmetic is exact (values < 2^24).
  - MLP: embedding renorm + W1a + b1 host-folded into G2[30,256]; one-hot
    (tissue) + freq share one PE stationary; W2 + sigmoid on PE/ACT.
"""

import os
import sys

import numpy as np

try:
    import concourse  # noqa: F401
except ImportError:  # pragma: no cover
    for _p in ("/opt/trn_rl_repo", "/root/.axon_site/_ro/trn_rl_repo"):
        if os.path.isdir(_p):
            sys.path.insert(0, _p)
            break

from contextlib import ExitStack

import concourse.bass as bass
import concourse.tile as tile
from concourse import bacc, mybir
from concourse import masks as cmasks
from concourse.bass_utils import run_bass_kernel_spmd

N_CORES = 8
B, L = 16384, 2048
RPC = B // N_CORES          # rows per core
NT = RPC // 128             # 128-row tiles per core (16)
NCH = L // 128              # 128-position chunks (16)
NBINS = 64                  # histogram bins 1..64 (bin 0 = padding, dropped)
NTIS = 30                   # tissue vocabulary
NFEAT = NTIS + NBINS        # 94 combined one-hot + freq features
NFEAT_PAD = 96
GROUP = 512                 # rows per PE moving block
NGRP = RPC // GROUP         # 4
Z_PE = 41                   # bins 1..Z_PE via PE mask-reduce
NDVE = 3                    # bins Z_PE+1..Z_PE+NDVE via DVE is_eq+accum
ABASE = Z_PE + NDVE         # 44: ACT bins ABASE+1..64
NACT = NBINS - ABASE        # 20
SCHW = 4                    # chunks per supertile
NSCH = NCH // SCHW          # 4

f32 = mybir.dt.float32
bf16 = mybir.dt.bfloat16
i32 = mybir.dt.int32
OP = mybir.AluOpType
AF = mybir.ActivationFunctionType

_CACHED = {}


def _build_program():
    nc = bacc.Bacc(
        "TRN2",
        target_bir_lowering=False,
        debug=False,
        enable_asserts=False,
        num_devices=N_CORES,
    )

    # rnat = transposed shard [L positions, RPC rows]; rna = row-major shard
    rnat = nc.dram_tensor("rnat", [L, RPC], i32, kind="ExternalInput").ap()
    rna = nc.dram_tensor("rna", [RPC, L], i32, kind="ExternalInput").ap()
    tis = nc.dram_tensor("tis", [128, NT], i32, kind="ExternalInput").ap()
    s0 = nc.dram_tensor("s0", [NFEAT_PAD, 128], f32, kind="ExternalInput").ap()
    s1 = nc.dram_tensor("s1", [NFEAT_PAD, 128], f32, kind="ExternalInput").ap()
    w2a = nc.dram_tensor("w2a", [128, 1], f32, kind="ExternalInput").ap()
    w2b = nc.dram_tensor("w2b", [128, 1], f32, kind="ExternalInput").ap()
    b2v = nc.dram_tensor("b2v", [1, 1], f32, kind="ExternalInput").ap()
    out = nc.dram_tensor("out", [NT, 128], f32, kind="ExternalOutput").ap()

    with tile.TileContext(nc) as tc, ExitStack() as ctx:
        const = ctx.enter_context(tc.tile_pool(name="const", bufs=1))

        ident = const.tile([128, 128], f32)
        cmasks.make_identity(nc, ident[:])
        # zsel: zeros except column 64 all-ones. Bin c's stationary is the
        # sliding view zsel[:, 65-c : 129-c] -> routes the mask-sum into
        # output partition row c-1.
        zsel = const.tile([128, 129], bf16)
        nc.vector.memset(zsel[:], 0.0)
        nc.vector.memset(zsel[:, 64:65], 1.0)

        iota_i = const.tile([128, 32], i32)
        nc.gpsimd.iota(iota_i[:], [[1, 32]], channel_multiplier=0)
        iota_f = const.tile([128, 32], f32)
        nc.vector.tensor_copy(iota_f[:], iota_i[:])

        s0_sb = const.tile([NFEAT_PAD, 128], f32)
        nc.sync.dma_start(s0_sb[:], s0)
        s1_sb = const.tile([NFEAT_PAD, 128], f32)
        nc.sync.dma_start(s1_sb[:], s1)
        w2a_sb = const.tile([128, 1], f32)
        nc.sync.dma_start(w2a_sb[:], w2a)
        w2b_sb = const.tile([128, 1], f32)
        nc.sync.dma_start(w2b_sb[:], w2b)
        b2_sb = const.tile([1, 1], f32)
        nc.sync.dma_start(b2_sb[:], b2v)
        tis_i = const.tile([128, NT], i32)
        nc.sync.dma_start(tis_i[:], tis)
        tis_f = const.tile([128, NT], f32)
        nc.vector.tensor_copy(tis_f[:], tis_i[:])

        # results staged across phases
        nacts = const.tile([128, NT * NACT], f32)   # ACT bins per tile
        naccs = const.tile([128, NT * NDVE], f32)   # DVE-accum bins per tile
        htt_all = const.tile([Z_PE, RPC], f32)      # PE histT staging
        # bias column k = -(ABASE + k) for the relu-T passes
        tbias = const.tile([128, NACT], f32)
        nc.vector.tensor_scalar(
            tbias[:], iota_f[:, 0:NACT], float(ABASE), -1.0, OP.add, OP.mult
        )

        rnat_pool = ctx.enter_context(tc.tile_pool(name="rnat", bufs=2))
        vbf_pool = ctx.enter_context(tc.tile_pool(name="vbf", bufs=2))
        mask_pool = ctx.enter_context(tc.tile_pool(name="mask", bufs=3))
        rowr_pool = ctx.enter_context(tc.tile_pool(name="rowr", bufs=2))
        rowb_pool = ctx.enter_context(tc.tile_pool(name="rowb", bufs=2))
        scr_pool = ctx.enter_context(tc.tile_pool(name="scr", bufs=2))
        hist_pool = ctx.enter_context(tc.tile_pool(name="hist", bufs=3))
        xoh_pool = ctx.enter_context(tc.tile_pool(name="xoh", bufs=3))
        small_pool = ctx.enter_context(tc.tile_pool(name="small", bufs=3))
        mlp_pool = ctx.enter_context(tc.tile_pool(name="mlp", bufs=3))

        # --- Phase 1: sweeps ---
        with tc.tile_pool(name="sweeppsum", bufs=1, space="PSUM") as sweep_psum:
            hps = []
            for bnk in range(NGRP):
                hb = sweep_psum.tile([64, GROUP], f32, tag=f"hb{bnk}")
                hps.append(hb)

            for sch in range(NSCH):
                rt = rnat_pool.tile([128, SCHW, RPC], i32)
                src = rnat[bass.ts(sch, 128 * SCHW), :].rearrange(
                    "(a b) c -> b a c", a=SCHW
                )
                nc.sync.dma_start(rt[:], src)
                vb = vbf_pool.tile([128, SCHW * RPC], bf16)
                nc.vector.tensor_copy(vb[:], rt[:].rearrange("b a c -> b (a c)"))
                for c in range(1, Z_PE + 1):
                    mask = mask_pool.tile([128, SCHW * RPC], bf16)
                    nc.vector.tensor_scalar(
                        mask[:], vb[:], float(c), None, OP.is_equal
                    )
                    for h in range(SCHW):
                        for g in range(NGRP):
                            nc.tensor.matmul(
                                hps[g][:],
                                zsel[:, 65 - c : 129 - c],
                                mask[
                                    :,
                                    h * RPC + g * GROUP : h * RPC + (g + 1) * GROUP,
                                ],
                                start=(sch == 0 and h == 0 and c == 1),
                                stop=(
                                    sch == NSCH - 1
                                    and h == SCHW - 1
                                    and c == Z_PE
                                ),
                            )

                # row-major work for SCHW row-tiles, interleaved for overlap
                for t in range(SCHW * sch, SCHW * (sch + 1)):
                    rtr = rowr_pool.tile([128, L], i32)
                    nc.sync.dma_start(rtr[:], rna[bass.ts(t, 128), :])
                    # DVE fused compare+accumulate bins (bf16 cast first:
                    # the accum-reduce variant rejects int32 inputs)
                    vbr = rowb_pool.tile([128, L], bf16)
                    nc.vector.tensor_copy(vbr[:], rtr[:])
                    for j in range(NDVE):
                        scr2 = scr_pool.tile([128, L], bf16, tag="scr2")
                        nc.vector.tensor_scalar(
                            scr2[:],
                            vbr[:],
                            float(Z_PE + 1 + j),
                            None,
                            OP.is_equal,
                            OP.add,
                            accum_out=naccs[:, t * NDVE + j : t * NDVE + j + 1],
                        )
                    # ACT relu-T passes (reads int32 directly)
                    tt = small_pool.tile([128, NACT + 2], f32, tag="tt")
                    nc.vector.memset(tt[:, NACT : NACT + 2], 0.0)
                    for k in range(NACT):
                        scr = scr_pool.tile([128, L], bf16, tag="scr")
                        nc.scalar.activation(
                            scr[:],
                            rtr[:],
                            AF.Relu,
                            bias=tbias[:, k : k + 1],
                            accum_out=tt[:, k : k + 1],
                        )
                    cdif = small_pool.tile([128, NACT + 1], f32, tag="cdif")
                    nc.vector.tensor_tensor(
                        cdif[:], tt[:, 0 : NACT + 1], tt[:, 1 : NACT + 2], OP.subtract
                    )
                    nc.vector.tensor_tensor(
                        nacts[:, t * NACT : (t + 1) * NACT],
                        cdif[:, 0:NACT],
                        cdif[:, 1 : NACT + 1],
                        OP.subtract,
                    )

            # drain PE hist PSUM to SBUF so the sweep banks can be reused
            for t in range(NT):
                g = t // 4
                sl = t % 4
                nc.vector.tensor_copy(
                    htt_all[:, bass.ts(t, 128)], hps[g][0:Z_PE, bass.ts(sl, 128)]
                )

        # --- Phase 2+3: assemble hist, normalize, MLP ---
        with (
            tc.tile_pool(name="trpsum", bufs=2, space="PSUM") as tr_psum,
            tc.tile_pool(name="mlppsum", bufs=2, space="PSUM") as mlp_psum,
        ):
            for t in range(NT):
                hps_t = tr_psum.tile([128, Z_PE], f32, tag="ht")
                nc.tensor.transpose(
                    hps_t[:], htt_all[:, bass.ts(t, 128)], ident[0:Z_PE, 0:Z_PE]
                )
                hist = hist_pool.tile([128, NBINS], f32)
                nc.vector.tensor_copy(hist[:, 0:Z_PE], hps_t[:])
                nc.vector.tensor_copy(
                    hist[:, Z_PE:ABASE], naccs[:, t * NDVE : (t + 1) * NDVE]
                )
                nc.vector.tensor_copy(
                    hist[:, ABASE:NBINS], nacts[:, t * NACT : (t + 1) * NACT]
                )

                xoh = xoh_pool.tile([128, NFEAT_PAD], f32)
                nc.vector.tensor_scalar(
                    xoh[:, 0:NTIS],
                    iota_f[:, 0:NTIS],
                    tis_f[:, t : t + 1],
                    None,
                    OP.is_equal,
                )
                nvec = small_pool.tile([128, 1], f32, tag="nvec")
                nc.vector.tensor_reduce(
                    nvec[:], hist[:], mybir.AxisListType.X, OP.add
                )
                rec = small_pool.tile([128, 1], f32, tag="rec")
                nc.vector.reciprocal(rec[:], nvec[:])
                nc.vector.tensor_scalar(
                    xoh[:, NTIS:NFEAT], hist[:], rec[:], None, OP.mult
                )
                nc.vector.memset(xoh[:, NFEAT:NFEAT_PAD], 0.0)

                xt_ps = tr_psum.tile([128, 128], f32, tag="xt")
                nc.tensor.transpose(xt_ps[0:NFEAT_PAD, :], xoh[:], ident[:])
                xt_sb = mlp_pool.tile([NFEAT_PAD, 128], f32, tag="xtsb")
                nc.vector.tensor_copy(xt_sb[:], xt_ps[0:NFEAT_PAD, :])

                hh = mlp_psum.tile([128, 256], f32, tag="hh")
                nc.tensor.matmul(
                    hh[:, 0:128], s0_sb[:], xt_sb[:], start=True, stop=False
                )
                nc.tensor.matmul(
                    hh[:, 128:256], s1_sb[:], xt_sb[:], start=False, stop=True
                )

                hr0 = mlp_pool.tile([128, 128], f32, tag="hr0")
                nc.vector.tensor_scalar(hr0[:], hh[:, 0:128], 0.0, None, OP.max)
                hr1 = mlp_pool.tile([128, 128], f32, tag="hr1")
                nc.vector.tensor_scalar(hr1[:], hh[:, 128:256], 0.0, None, OP.max)

                yps = mlp_psum.tile([1, 128], f32, tag="y")
                nc.tensor.matmul(yps[:], w2a_sb[:], hr0[:], start=True, stop=False)
                nc.tensor.matmul(yps[:], w2b_sb[:], hr1[:], start=False, stop=True)

                ysb = mlp_pool.tile([1, 128], f32, tag="ysb")
                nc.scalar.activation(ysb[:], yps[:], AF.Sigmoid, bias=b2_sb[:])
                nc.sync.dma_start(out[t : t + 1, :], ysb[:])

    nc.compile()
    return nc


def _host_tables(emb_table, W1, b1, W2, b2):
    emb_table = np.asarray(emb_table, np.float32)
    W1 = np.asarray(W1, np.float32)
    b1 = np.asarray(b1, np.float32)
    W2 = np.asarray(W2, np.float32)
    b2 = np.asarray(b2, np.float32)

    norm = np.linalg.norm(emb_table, axis=1, keepdims=True)
    scale = np.where(norm > 1.0, 1.0 / (norm + 1e-7), 1.0).astype(np.float32)
    embr = emb_table * scale                      # [30, 64]

    W1a = W1[:, :NBINS]                           # [256, 64] (emb features)
    W1b = W1[:, NBINS:]                           # [256, 64] (freq features)
    G2 = embr @ W1a.T + b1[None, :]               # [30, 256]

    s_halves = []
    for h in range(2):
        S = np.zeros((NFEAT_PAD, 128), np.float32)
        S[0:NTIS, :] = G2[:, 128 * h : 128 * (h + 1)]
        S[NTIS:NFEAT, :] = W1b[128 * h : 128 * (h + 1), :].T
        s_halves.append(S)
    w2a = np.ascontiguousarray(W2[0, 0:128].reshape(128, 1))
    w2b = np.ascontiguousarray(W2[0, 128:256].reshape(128, 1))
    b2v = np.array([[float(b2.reshape(-1)[0])]], np.float32)
    return s_halves[0], s_halves[1], w2a, w2b, b2v


def build_in_maps(rna_data, tissue_id, emb_table, W1, b1, W2, b2):
    rna_data = np.asarray(rna_data, np.int32)
    tissue_id = np.asarray(tissue_id, np.int32).reshape(-1)
    s0, s1, w2a, w2b, b2v = _host_tables(emb_table, W1, b1, W2, b2)

    in_maps = []
    for c in range(N_CORES):
        rsh = np.ascontiguousarray(rna_data[c * RPC : (c + 1) * RPC])
        tsh = tissue_id[c * RPC : (c + 1) * RPC]
        tis_m = np.ascontiguousarray(tsh.reshape(NT, 128).T.astype(np.int32))
        in_maps.append(
            {
                "rnat": np.ascontiguousarray(rsh.T),
                "rna": rsh,
                "tis": tis_m,
                "s0": s0,
                "s1": s1,
                "w2a": w2a,
                "w2b": w2b,
                "b2v": b2v,
            }
        )
    return in_maps


def get_program():
    if "nc" not in _CACHED:
        _CACHED["nc"] = _build_program()
    return _CACHED["nc"]


def kernel(rna_data, tissue_id, emb_table, W1, b1, W2, b2, **run_kwargs):
    nc = get_program()
    in_maps = build_in_maps(rna_data, tissue_id, emb_table, W1, b1, W2, b2)
    res = run_bass_kernel_spmd(nc, in_maps, list(range(N_CORES)), **run_kwargs)
    outs = [res.results[c]["out"].reshape(RPC) for c in range(N_CORES)]
    full = np.concatenate(outs).reshape(B, 1).astype(np.float32)
    if run_kwargs:
        return full, res
    return full


# revision 29
# speedup vs baseline: 1.1977x; 1.0480x over previous
"""Trainium2 Bass kernel: per-row codon histogram + tissue embedding + 2-layer MLP.

Model (matches the jax reference):
  freq[b]  = hist(rna_data[b], bins 1..64) / n_nonzero[b]
  emb[b]   = renorm(emb_table)[tissue_id[b]]
  x        = [emb | freq]  (128 features)
  h        = relu(x @ W1.T + b1)
  out[b]   = sigmoid(h @ W2.T + b2)

Mapping (v4) — three-way bin split to balance engines:
  - bins 1..41  (PE):  data staged transposed [positions, rows]; DVE builds
    (data == c) bf16 masks at 4x; PE reduces masks along positions via a
    sliding-window selector stationary into PSUM row c-1, accumulating
    across position chunks. One PSUM bank per 512-row group.
  - bins 42..44 (DVE): fused is_equal+accumulate on row-major data (1x).
  - bins 45..64 (ACT): hockey-stick trick on row-major data: T(c) =
    sum relu(v - c) per row via activation accum; exact counts via second
    differences. fp32 arithmetic is exact (values < 2^24).
  - MLP: embedding renorm + W1a + b1 host-folded into G2[30,256]; one-hot
    (tissue) + freq share one PE stationary; W2 + sigmoid on PE/ACT.
"""

import os
import sys

import numpy as np

try:
    import concourse  # noqa: F401
except ImportError:  # pragma: no cover
    for _p in ("/opt/trn_rl_repo", "/root/.axon_site/_ro/trn_rl_repo"):
        if os.path.isdir(_p):
            sys.path.insert(0, _p)
            break

from contextlib import ExitStack

import concourse.bass as bass
import concourse.tile as tile
from concourse import bacc, mybir
from concourse import masks as cmasks
from concourse.bass_utils import run_bass_kernel_spmd

N_CORES = 8
B, L = 16384, 2048
RPC = B // N_CORES          # rows per core
NT = RPC // 128             # 128-row tiles per core (16)
NCH = L // 128              # 128-position chunks (16)
NBINS = 64                  # histogram bins 1..64 (bin 0 = padding, dropped)
NTIS = 30                   # tissue vocabulary
NFEAT = NTIS + NBINS        # 94 combined one-hot + freq features
NFEAT_PAD = 96
GROUP = 512                 # rows per PE moving block
NGRP = RPC // GROUP         # 4
Z_PE = 42                   # bins 1..Z_PE via PE mask-reduce
NDVE = 3                    # bins Z_PE+1..Z_PE+NDVE via DVE is_eq+accum
ABASE = Z_PE + NDVE         # 44: ACT bins ABASE+1..64
NACT = NBINS - ABASE        # 20
SCHW = 4                    # chunks per supertile
NSCH = NCH // SCHW          # 4

f32 = mybir.dt.float32
bf16 = mybir.dt.bfloat16
i32 = mybir.dt.int32
OP = mybir.AluOpType
AF = mybir.ActivationFunctionType

_CACHED = {}


def _build_program():
    nc = bacc.Bacc(
        "TRN2",
        target_bir_lowering=False,
        debug=False,
        enable_asserts=False,
        num_devices=N_CORES,
    )

    # rnat = transposed shard [L positions, RPC rows]; rna = row-major shard
    rnat = nc.dram_tensor("rnat", [L, RPC], i32, kind="ExternalInput").ap()
    rna = nc.dram_tensor("rna", [RPC, L], i32, kind="ExternalInput").ap()
    tis = nc.dram_tensor("tis", [128, NT], i32, kind="ExternalInput").ap()
    s0 = nc.dram_tensor("s0", [NFEAT_PAD, 128], f32, kind="ExternalInput").ap()
    s1 = nc.dram_tensor("s1", [NFEAT_PAD, 128], f32, kind="ExternalInput").ap()
    w2a = nc.dram_tensor("w2a", [128, 1], f32, kind="ExternalInput").ap()
    w2b = nc.dram_tensor("w2b", [128, 1], f32, kind="ExternalInput").ap()
    b2v = nc.dram_tensor("b2v", [1, 1], f32, kind="ExternalInput").ap()
    out = nc.dram_tensor("out", [NT, 128], f32, kind="ExternalOutput").ap()

    with tile.TileContext(nc) as tc, ExitStack() as ctx:
        const = ctx.enter_context(tc.tile_pool(name="const", bufs=1))

        ident = const.tile([128, 128], f32)
        cmasks.make_identity(nc, ident[:])
        # zsel: zeros except column 64 all-ones. Bin c's stationary is the
        # sliding view zsel[:, 65-c : 129-c] -> routes the mask-sum into
        # output partition row c-1.
        zsel = const.tile([128, 129], bf16)
        nc.vector.memset(zsel[:], 0.0)
        nc.vector.memset(zsel[:, 64:65], 1.0)

        iota_i = const.tile([128, 32], i32)
        nc.gpsimd.iota(iota_i[:], [[1, 32]], channel_multiplier=0)
        iota_f = const.tile([128, 32], f32)
        nc.vector.tensor_copy(iota_f[:], iota_i[:])

        s0_sb = const.tile([NFEAT_PAD, 128], f32)
        nc.sync.dma_start(s0_sb[:], s0)
        s1_sb = const.tile([NFEAT_PAD, 128], f32)
        nc.sync.dma_start(s1_sb[:], s1)
        w2a_sb = const.tile([128, 1], f32)
        nc.sync.dma_start(w2a_sb[:], w2a)
        w2b_sb = const.tile([128, 1], f32)
        nc.sync.dma_start(w2b_sb[:], w2b)
        b2_sb = const.tile([1, 1], f32)
        nc.sync.dma_start(b2_sb[:], b2v)
        tis_i = const.tile([128, NT], i32)
        nc.sync.dma_start(tis_i[:], tis)
        tis_f = const.tile([128, NT], f32)
        nc.vector.tensor_copy(tis_f[:], tis_i[:])

        # results staged across phases
        nacts = const.tile([128, NT * NACT], f32)   # ACT bins per tile
        naccs = const.tile([128, NT * NDVE], f32)   # DVE-accum bins per tile
        htt_all = const.tile([Z_PE, RPC], f32)      # PE histT staging
        # bias column k = -(ABASE + k) for the relu-T passes
        tbias = const.tile([128, NACT], f32)
        nc.vector.tensor_scalar(
            tbias[:], iota_f[:, 0:NACT], float(ABASE), -1.0, OP.add, OP.mult
        )

        rnat_pool = ctx.enter_context(tc.tile_pool(name="rnat", bufs=2))
        vbf_pool = ctx.enter_context(tc.tile_pool(name="vbf", bufs=2))
        mask_pool = ctx.enter_context(tc.tile_pool(name="mask", bufs=3))
        rowr_pool = ctx.enter_context(tc.tile_pool(name="rowr", bufs=2))
        rowb_pool = ctx.enter_context(tc.tile_pool(name="rowb", bufs=2))
        scr_pool = ctx.enter_context(tc.tile_pool(name="scr", bufs=2))
        hist_pool = ctx.enter_context(tc.tile_pool(name="hist", bufs=3))
        xoh_pool = ctx.enter_context(tc.tile_pool(name="xoh", bufs=3))
        small_pool = ctx.enter_context(tc.tile_pool(name="small", bufs=3))
        mlp_pool = ctx.enter_context(tc.tile_pool(name="mlp", bufs=3))

        # --- Phase 1: sweeps ---
        with tc.tile_pool(name="sweeppsum", bufs=1, space="PSUM") as sweep_psum:
            hps = []
            for bnk in range(NGRP):
                hb = sweep_psum.tile([64, GROUP], f32, tag=f"hb{bnk}")
                hps.append(hb)

            for sch in range(NSCH):
                rt = rnat_pool.tile([128, SCHW, RPC], i32)
                src = rnat[bass.ts(sch, 128 * SCHW), :].rearrange(
                    "(a b) c -> b a c", a=SCHW
                )
                nc.sync.dma_start(rt[:], src)
                vb = vbf_pool.tile([128, SCHW * RPC], bf16)
                nc.vector.tensor_copy(vb[:], rt[:].rearrange("b a c -> b (a c)"))
                for c in range(1, Z_PE + 1):
                    mask = mask_pool.tile([128, SCHW * RPC], bf16)
                    nc.vector.tensor_scalar(
                        mask[:], vb[:], float(c), None, OP.is_equal
                    )
                    for h in range(SCHW):
                        for g in range(NGRP):
                            nc.tensor.matmul(
                                hps[g][:],
                                zsel[:, 65 - c : 129 - c],
                                mask[
                                    :,
                                    h * RPC + g * GROUP : h * RPC + (g + 1) * GROUP,
                                ],
                                start=(sch == 0 and h == 0 and c == 1),
                                stop=(
                                    sch == NSCH - 1
                                    and h == SCHW - 1
                                    and c == Z_PE
                                ),
                            )

                # row-major work for SCHW row-tiles, interleaved for overlap
                for t in range(SCHW * sch, SCHW * (sch + 1)):
                    rtr = rowr_pool.tile([128, L], i32)
                    nc.sync.dma_start(rtr[:], rna[bass.ts(t, 128), :])
                    # DVE fused compare+accumulate bins (bf16 cast first:
                    # the accum-reduce variant rejects int32 inputs)
                    vbr = rowb_pool.tile([128, L], bf16)
                    nc.vector.tensor_copy(vbr[:], rtr[:])
                    for j in range(NDVE):
                        scr2 = scr_pool.tile([128, L], bf16, tag="scr2")
                        nc.vector.tensor_scalar(
                            scr2[:],
                            vbr[:],
                            float(Z_PE + 1 + j),
                            None,
                            OP.is_equal,
                            OP.add,
                            accum_out=naccs[:, t * NDVE + j : t * NDVE + j + 1],
                        )
                    # ACT relu-T passes (reads int32 directly)
                    tt = small_pool.tile([128, NACT + 2], f32, tag="tt")
                    nc.vector.memset(tt[:, NACT : NACT + 2], 0.0)
                    for k in range(NACT):
                        scr = scr_pool.tile([128, L], bf16, tag="scr")
                        nc.scalar.activation(
                            scr[:],
                            vbr[:],
                            AF.Relu,
                            bias=tbias[:, k : k + 1],
                            accum_out=tt[:, k : k + 1],
                        )
                    cdif = small_pool.tile([128, NACT + 1], f32, tag="cdif")
                    nc.vector.tensor_tensor(
                        cdif[:], tt[:, 0 : NACT + 1], tt[:, 1 : NACT + 2], OP.subtract
                    )
                    nc.vector.tensor_tensor(
                        nacts[:, t * NACT : (t + 1) * NACT],
                        cdif[:, 0:NACT],
                        cdif[:, 1 : NACT + 1],
                        OP.subtract,
                    )

            # drain PE hist PSUM to SBUF so the sweep banks can be reused
            for t in range(NT):
                g = t // 4
                sl = t % 4
                nc.vector.tensor_copy(
                    htt_all[:, bass.ts(t, 128)], hps[g][0:Z_PE, bass.ts(sl, 128)]
                )

        # --- Phase 2+3: assemble hist, normalize, MLP ---
        with (
            tc.tile_pool(name="trpsum", bufs=2, space="PSUM") as tr_psum,
            tc.tile_pool(name="mlppsum", bufs=2, space="PSUM") as mlp_psum,
        ):
            for t in range(NT):
                hps_t = tr_psum.tile([128, Z_PE], f32, tag="ht")
                nc.tensor.transpose(
                    hps_t[:], htt_all[:, bass.ts(t, 128)], ident[0:Z_PE, 0:Z_PE]
                )
                hist = hist_pool.tile([128, NBINS], f32)
                nc.vector.tensor_copy(hist[:, 0:Z_PE], hps_t[:])
                nc.vector.tensor_copy(
                    hist[:, Z_PE:ABASE], naccs[:, t * NDVE : (t + 1) * NDVE]
                )
                nc.vector.tensor_copy(
                    hist[:, ABASE:NBINS], nacts[:, t * NACT : (t + 1) * NACT]
                )

                xoh = xoh_pool.tile([128, NFEAT_PAD], f32)
                nc.vector.tensor_scalar(
                    xoh[:, 0:NTIS],
                    iota_f[:, 0:NTIS],
                    tis_f[:, t : t + 1],
                    None,
                    OP.is_equal,
                )
                nvec = small_pool.tile([128, 1], f32, tag="nvec")
                nc.vector.tensor_reduce(
                    nvec[:], hist[:], mybir.AxisListType.X, OP.add
                )
                rec = small_pool.tile([128, 1], f32, tag="rec")
                nc.vector.reciprocal(rec[:], nvec[:])
                nc.vector.tensor_scalar(
                    xoh[:, NTIS:NFEAT], hist[:], rec[:], None, OP.mult
                )
                nc.vector.memset(xoh[:, NFEAT:NFEAT_PAD], 0.0)

                xt_ps = tr_psum.tile([128, 128], f32, tag="xt")
                nc.tensor.transpose(xt_ps[0:NFEAT_PAD, :], xoh[:], ident[:])
                xt_sb = mlp_pool.tile([NFEAT_PAD, 128], f32, tag="xtsb")
                nc.vector.tensor_copy(xt_sb[:], xt_ps[0:NFEAT_PAD, :])

                hh = mlp_psum.tile([128, 256], f32, tag="hh")
                nc.tensor.matmul(
                    hh[:, 0:128], s0_sb[:], xt_sb[:], start=True, stop=False
                )
                nc.tensor.matmul(
                    hh[:, 128:256], s1_sb[:], xt_sb[:], start=False, stop=True
                )

                hr0 = mlp_pool.tile([128, 128], f32, tag="hr0")
                nc.vector.tensor_scalar(hr0[:], hh[:, 0:128], 0.0, None, OP.max)
                hr1 = mlp_pool.tile([128, 128], f32, tag="hr1")
                nc.vector.tensor_scalar(hr1[:], hh[:, 128:256], 0.0, None, OP.max)

                yps = mlp_psum.tile([1, 128], f32, tag="y")
                nc.tensor.matmul(yps[:], w2a_sb[:], hr0[:], start=True, stop=False)
                nc.tensor.matmul(yps[:], w2b_sb[:], hr1[:], start=False, stop=True)

                ysb = mlp_pool.tile([1, 128], f32, tag="ysb")
                nc.scalar.activation(ysb[:], yps[:], AF.Sigmoid, bias=b2_sb[:])
                nc.sync.dma_start(out[t : t + 1, :], ysb[:])

    nc.compile()
    return nc


def _host_tables(emb_table, W1, b1, W2, b2):
    emb_table = np.asarray(emb_table, np.float32)
    W1 = np.asarray(W1, np.float32)
    b1 = np.asarray(b1, np.float32)
    W2 = np.asarray(W2, np.float32)
    b2 = np.asarray(b2, np.float32)

    norm = np.linalg.norm(emb_table, axis=1, keepdims=True)
    scale = np.where(norm > 1.0, 1.0 / (norm + 1e-7), 1.0).astype(np.float32)
    embr = emb_table * scale                      # [30, 64]

    W1a = W1[:, :NBINS]                           # [256, 64] (emb features)
    W1b = W1[:, NBINS:]                           # [256, 64] (freq features)
    G2 = embr @ W1a.T + b1[None, :]               # [30, 256]

    s_halves = []
    for h in range(2):
        S = np.zeros((NFEAT_PAD, 128), np.float32)
        S[0:NTIS, :] = G2[:, 128 * h : 128 * (h + 1)]
        S[NTIS:NFEAT, :] = W1b[128 * h : 128 * (h + 1), :].T
        s_halves.append(S)
    w2a = np.ascontiguousarray(W2[0, 0:128].reshape(128, 1))
    w2b = np.ascontiguousarray(W2[0, 128:256].reshape(128, 1))
    b2v = np.array([[float(b2.reshape(-1)[0])]], np.float32)
    return s_halves[0], s_halves[1], w2a, w2b, b2v


def build_in_maps(rna_data, tissue_id, emb_table, W1, b1, W2, b2):
    rna_data = np.asarray(rna_data, np.int32)
    tissue_id = np.asarray(tissue_id, np.int32).reshape(-1)
    s0, s1, w2a, w2b, b2v = _host_tables(emb_table, W1, b1, W2, b2)

    in_maps = []
    for c in range(N_CORES):
        rsh = np.ascontiguousarray(rna_data[c * RPC : (c + 1) * RPC])
        tsh = tissue_id[c * RPC : (c + 1) * RPC]
        tis_m = np.ascontiguousarray(tsh.reshape(NT, 128).T.astype(np.int32))
        in_maps.append(
            {
                "rnat": np.ascontiguousarray(rsh.T),
                "rna": rsh,
                "tis": tis_m,
                "s0": s0,
                "s1": s1,
                "w2a": w2a,
                "w2b": w2b,
                "b2v": b2v,
            }
        )
    return in_maps


def get_program():
    if "nc" not in _CACHED:
        _CACHED["nc"] = _build_program()
    return _CACHED["nc"]


def kernel(rna_data, tissue_id, emb_table, W1, b1, W2, b2, **run_kwargs):
    nc = get_program()
    in_maps = build_in_maps(rna_data, tissue_id, emb_table, W1, b1, W2, b2)
    res = run_bass_kernel_spmd(nc, in_maps, list(range(N_CORES)), **run_kwargs)
    outs = [res.results[c]["out"].reshape(RPC) for c in range(N_CORES)]
    full = np.concatenate(outs).reshape(B, 1).astype(np.float32)
    if run_kwargs:
        return full, res
    return full
